# revision 12
# baseline (speedup 1.0000x reference)
"""GAT (3-layer graph attention + final linear) Trainium2 Bass kernel.

Problem: B=4 graphs, N=2048 atoms, D=128, H=256.
  per layer: h = relu(x @ W.T + b); e_ij = leaky_relu(f1_i + f2_j, 0.01)
  masked by adj; att = softmax_j(e); x = x + att @ h.
  final: relu(x @ Wt.T + bt).

Sharding: 8 cores; core c -> (graph b=c//2, row-half s=c%2 of the NxN
attention). Each core computes attention for its 1024 rows (i) over all
2048 columns (j), in a core-local [local|remote] column layout: j tiles
0-7 are the core's OWN rows (h computed locally), 8-15 the partner's
(via a pair AllGather that overlaps the local tiles). The host permutes
the mask and xT inputs to match, so the program is SPMD-uniform.

Key structure (all matmuls bf16, logits fused into ACT):
  - adj transposed on the HOST into a bf16 0/1 mask, [j, i] layout,
    j-tiles permuted local-first per core.
  - logits built inside the activation pass: t = Prelu(f1bc + f2col_j,
    alpha=.01), q = Exp(t); both share one ACT table set. No row-max
    needed: logits are bounded (~36) so f32 exp is safe.
  - mask applied multiplicatively after exp on DVE (bf16).
  - aggregation transposed: psAT[d,i] += hnat_j^T @ p_j; row sums via a
    ones-column matmul into psS[1,i]. Normalize + residual stay in the
    transposed layout: 1/s computed in a [128,8] column shape (DVE
    reciprocal is ~6.4ns/elem, so never on [1,N] rows), broadcast back
    over partitions with ones-matmuls.
  - AllGather import is SPMD-uniform and runs on the idle GpSimd engine:
    remote = (hg0 + hg1) - local, exact in f32 for bf16 inputs.
  - output written transposed [H, NS]; host transposes back.
"""

import numpy as np
import ml_dtypes

import concourse.bass as bass
import concourse.mybir as mybir
import concourse.tile as tile
from concourse import masks
from concourse.bass_utils import run_bass_kernel_spmd

P = 128
F32 = mybir.dt.float32
BF16 = mybir.dt.bfloat16
AF = mybir.ActivationFunctionType
OP = mybir.AluOpType


def _legalize_waits(nc, dma_limit=1, engine_limit=1):
    """Walrus can encode only 1 sem wait on a DMA instruction and ~2 on an
    engine instruction. Move excess waits onto standalone EventSemaphore
    instructions (1 wait each) inserted just before the offender on the
    same engine."""
    counter = [0]

    def split(ins):
        si = ins.sync_info
        if si is None:
            return None
        limit = dma_limit if type(ins).__name__.startswith("InstDMA") \
            else engine_limit
        waits = list(si.on_wait)
        if len(waits) <= limit:
            return None
        keep = waits[-limit:] if limit > 0 else []
        extra = waits[:-limit] if limit > 0 else waits
        evs = []
        for w in extra:
            counter[0] += 1
            evs.append(mybir.InstEventSemaphore(
                name=f"evsplit{counter[0]}", engine=ins.engine,
                sync_info=mybir.SyncInfo(on_wait=[w], on_update=[])))
        ins.sync_info = mybir.SyncInfo(on_wait=keep,
                                       on_update=list(si.on_update))
        return evs

    for f in nc.m.functions:
        for blk in f.blocks:
            new_list = []
            changed = False
            for ins in blk.instructions:
                evs = split(ins)
                if evs:
                    new_list.extend(evs)
                    changed = True
                new_list.append(ins)
            if changed:
                blk.instructions = new_list


def build_gat_nc(N, NS, D, H, num_cores, pair_groups, nlayers=3,
                 legalize=True):
    assert D == P and NS % 512 == 0 and N % 512 == 0
    nj = N // P        # j tiles (16)
    njl = nj // 2      # local j tiles (8)
    nch = NS // 512    # 512-chunks in the i shard (2)
    nit = NS // P      # i tiles (8)
    nH = H // P

    nc = bass.Bass("TRN2", target_bir_lowering=False, debug=False,
                   num_devices=num_cores)

    # ---- I/O ----
    xTsb_in = nc.dram_tensor("xTsb", [P, NS], BF16, kind="ExternalInput")
    xTs_in = nc.dram_tensor("xTs", [P, NS], F32, kind="ExternalInput")
    xTb_in = nc.dram_tensor("xTb", [P, N], BF16, kind="ExternalInput")
    mask_in = nc.dram_tensor("maskTb", [N, NS], BF16, kind="ExternalInput")
    WT_in = [nc.dram_tensor(f"WT{l}", [D, D], BF16, kind="ExternalInput")
             for l in range(nlayers)]
    bv_in = [nc.dram_tensor(f"bv{l}", [D, 1], F32, kind="ExternalInput")
             for l in range(nlayers)]
    av_in = [nc.dram_tensor(f"av{l}", [D, 2], BF16, kind="ExternalInput")
             for l in range(nlayers)]
    WtT_in = nc.dram_tensor("WtT", [D, H], BF16, kind="ExternalInput")
    btp_in = nc.dram_tensor("btp", [P, nH], F32, kind="ExternalInput")
    out_ext = nc.dram_tensor("outT_s", [H, NS], F32, kind="ExternalOutput")

    # DRAM bounce buffers for the pair AllGather of h shards (layers 1..)
    ag_in = [None] + [nc.dram_tensor(f"ag_in{l}", [P, NS], BF16)
                      for l in range(1, nlayers)]
    ag_out = [None] + [nc.dram_tensor(f"ag_out{l}", [2 * P, NS], BF16)
                       for l in range(1, nlayers)]

    with tile.TileContext(nc) as tc:
        import contextlib
        ctx = contextlib.ExitStack()
        with ctx:
            persist = ctx.enter_context(tc.tile_pool(name="persist", bufs=1))
            htp = ctx.enter_context(tc.tile_pool(name="htp", bufs=2))
            hgp = ctx.enter_context(tc.tile_pool(name="hgp", bufs=2))
            xtp = ctx.enter_context(tc.tile_pool(name="xtp", bufs=2))
            fbp = ctx.enter_context(tc.tile_pool(name="fbp", bufs=2))
            qp = ctx.enter_context(tc.tile_pool(name="qp", bufs=2))
            hnp = ctx.enter_context(tc.tile_pool(name="hnp", bufs=2))
            smallp = ctx.enter_context(tc.tile_pool(name="smallp", bufs=2))
            ocp = ctx.enter_context(tc.tile_pool(name="ocp", bufs=2))
            pe_pool = ctx.enter_context(
                tc.tile_pool(name="pe_pool", bufs=4, space="PSUM"))
            attp = ctx.enter_context(
                tc.tile_pool(name="attp", bufs=1, space="PSUM"))
            spp = ctx.enter_context(
                tc.tile_pool(name="spp", bufs=1, space="PSUM"))

            identb = persist.tile([P, P], BF16)
            masks.make_identity(nc, identb[:])
            identf = persist.tile([P, P], F32)
            masks.make_identity(nc, identf[:])
            onescol = persist.tile([P, 1], BF16)
            nc.vector.memset(onescol[:], 1.0)
            onesrowb = persist.tile([1, P], BF16)
            nc.vector.memset(onesrowb[:], 1.0)

            # ---- weights (critical-path inputs first) ----
            WT = [persist.tile([D, D], BF16, name=f"WT{l}", tag=f"WT{l}")
                  for l in range(nlayers)]
            bv = [persist.tile([D, 1], F32, name=f"bv{l}", tag=f"bv{l}")
                  for l in range(nlayers)]
            av = [persist.tile([D, 2], BF16, name=f"av{l}", tag=f"av{l}")
                  for l in range(nlayers)]
            WtTt = persist.tile([D, H], BF16)
            btpt = persist.tile([P, nH], F32)
            nc.sync.dma_start(WT[0][:], WT_in[0].ap())
            nc.sync.dma_start(bv[0][:], bv_in[0].ap())
            nc.sync.dma_start(av[0][:], av_in[0].ap())
            xTb = persist.tile([P, N], BF16)
            nc.sync.dma_start(xTb[:, 0:NS], xTb_in.ap()[:, 0:NS])
            nc.sync.dma_start(xTb[:, NS:N], xTb_in.ap()[:, NS:N])
            for l in range(1, nlayers):
                nc.sync.dma_start(WT[l][:], WT_in[l].ap())
                nc.sync.dma_start(bv[l][:], bv_in[l].ap())
                nc.sync.dma_start(av[l][:], av_in[l].ap())
            nc.sync.dma_start(WtTt[:], WtT_in.ap())
            nc.sync.dma_start(btpt[:], btp_in.ap())

            # ---- initial x state (transposed bf16 + f32 residual) ----
            xTsb = xtp.tile([P, NS], BF16, name="xTsb0", tag="xTsb")
            nc.sync.dma_start(xTsb[:], xTsb_in.ap())
            xTs = xtp.tile([P, NS], F32, name="xTs0", tag="xTs")
            nc.sync.dma_start(xTs[:], xTs_in.ap())

            # ---- adjacency mask tiles (bf16 0/1, [j, i], local-first) ----
            maskM = [persist.tile([P, NS], BF16, name=f"maskM{j}",
                                  tag=f"maskM{j}") for j in range(nj)]
            for j in range(nj):
                nc.sync.dma_start(maskM[j][:],
                                  mask_in.ap()[j * P:(j + 1) * P, :])

            # deferred off-critical-path emission (residual f32 add)
            pending = []

            for l in range(nlayers):
                last = l == nlayers - 1
                hT = htp.tile([P, N], BF16, name=f"hT{l}", tag="hT")
                # -- local h -> hT[:, 0:NS] --
                loc_src = xTb if l == 0 else xTsb
                for ch in range(nch):
                    sl = slice(ch * 512, (ch + 1) * 512)
                    ps = pe_pool.tile([P, 512], F32, name=f"hps{l}_{ch}",
                                      tag="pe")
                    nc.tensor.matmul(ps[:], WT[l][:], loc_src[:, sl],
                                     start=True, stop=True)
                    nc.vector.tensor_scalar(hT[:, sl], ps[:], bv[l][:],
                                            0.0, OP.add, OP.max)
                if l > 0:
                    nc.gpsimd.dma_start(ag_in[l].ap(), hT[:, 0:NS])
                    nc.gpsimd.collective_compute(
                        "AllGather", OP.bypass, replica_groups=pair_groups,
                        ins=[ag_in[l].ap()], outs=[ag_out[l].ap()])
                    # deferred residual f32 add rides in the collective
                    # shadow on the gpsimd queue
                    for fn in pending:
                        fn()
                    pending = []
                    hg0 = hgp.tile([P, NS], BF16, name=f"hg0_{l}",
                                   tag="hg0")
                    hg1 = hgp.tile([P, NS], BF16, name=f"hg1_{l}",
                                   tag="hg1")
                    nc.gpsimd.dma_start(hg0[:], ag_out[l].ap()[0:P, :])
                    nc.gpsimd.dma_start(hg1[:], ag_out[l].ap()[P:2 * P, :])

                # -- f1 over shard rows (from local h half), bf16 --
                f1row = smallp.tile([1, NS], BF16, name=f"f1row{l}",
                                    tag="f1row")
                for ch in range(nch):
                    sl = slice(ch * 512, (ch + 1) * 512)
                    psf = pe_pool.tile([2, 512], F32, name=f"fps{l}_{ch}",
                                       tag="pe")
                    nc.tensor.matmul(psf[:], av[l][:], hT[:, sl],
                                     start=True, stop=True)
                    nc.vector.tensor_copy(f1row[0:1, sl], psf[0:1, :])
                f1bc = fbp.tile([P, NS], BF16, name=f"f1bc{l}", tag="f1bc")
                for ch in range(nch):
                    sl = slice(ch * 512, (ch + 1) * 512)
                    ps = pe_pool.tile([P, 512], F32, name=f"bcf{l}_{ch}",
                                      tag="pe")
                    nc.tensor.matmul(ps[:], onesrowb[:], f1row[0:1, sl],
                                     start=True, stop=True)
                    nc.vector.tensor_copy(f1bc[:, sl], ps[:])

                # -- remote h for layer 0 (computed locally) --
                if l == 0:
                    for ch in range(nch):
                        sl = slice(NS + ch * 512, NS + (ch + 1) * 512)
                        ps = pe_pool.tile([P, 512], F32,
                                          name=f"hrps{l}_{ch}", tag="pe")
                        nc.tensor.matmul(ps[:], WT[l][:], xTb[:, sl],
                                         start=True, stop=True)
                        nc.vector.tensor_scalar(hT[:, sl], ps[:], bv[l][:],
                                                0.0, OP.add, OP.max)

                # -- per-j-tile [f1col, f2col] and natural-layout h --
                f2c = [None] * (nj // 4)
                hnatg = [None] * (nj // 4)

                def prep_group(g, l=l, hT=hT, f2c=f2c, hnatg=hnatg):
                    psc = pe_pool.tile([P, 8], F32, name=f"psc{l}_{g}",
                                       tag="pe")
                    for q in range(4):
                        t = g * 4 + q
                        nc.tensor.matmul(psc[:, 2 * q:2 * q + 2],
                                         hT[:, t * P:(t + 1) * P], av[l][:],
                                         start=True, stop=True)
                    fc = smallp.tile([P, 8], F32, name=f"f2c{l}_{g}",
                                     tag=f"f2c{g}")
                    nc.vector.tensor_copy(fc[:], psc[:])
                    f2c[g] = fc
                    pst = pe_pool.tile([P, 512], BF16, name=f"htp{l}_{g}",
                                       tag="pe")
                    for q in range(4):
                        t = g * 4 + q
                        nc.tensor.transpose(pst[:, q * P:(q + 1) * P],
                                            hT[:, t * P:(t + 1) * P],
                                            identb[:])
                    hn = hnp.tile([P, 512], BF16, name=f"hng{l}_{g}",
                                  tag=f"hng{g}")
                    nc.vector.tensor_copy(hn[:], pst[:])
                    hnatg[g] = hn

                prep_group(0)
                prep_group(1)
                if l == 0:
                    prep_group(2)
                    prep_group(3)

                # ---- attention: logits on ACT, mask on DVE, agg on PE ----
                psAT = attp.tile([P, NS], F32, name=f"psAT{l}", tag="att")
                psS = spp.tile([1, NS], F32, name=f"psS{l}", tag="s")

                def att_tile(t, l=l, psAT=psAT, psS=psS, f2c=f2c,
                             hnatg=hnatg, f1bc=f1bc):
                    g, q = t // 4, t % 4
                    tf = qp.tile([P, NS], F32, name=f"tf{l}_{t}", tag="tf")
                    nc.scalar.activation(tf[:], f1bc[:], AF.Prelu,
                                         bias=f2c[g][:, 2 * q + 1:2 * q + 2],
                                         scale=1.0, alpha=0.01)
                    qb = qp.tile([P, NS], BF16, name=f"qb{l}_{t}", tag="qb")
                    nc.scalar.activation(qb[:], tf[:], AF.Exp)
                    pb = qp.tile([P, NS], BF16, name=f"pb{l}_{t}", tag="pb")
                    nc.vector.tensor_tensor(pb[:], qb[:], maskM[t][:],
                                            OP.mult)
                    # on the last tile close the row-sum bank first so the
                    # reciprocal chain can start before the last agg matmul
                    mm = []
                    for ch in range(nch):
                        sl = slice(ch * 512, (ch + 1) * 512)
                        mm.append((psAT, hnatg[g][:, q * P:(q + 1) * P],
                                   pb[:, sl], sl, False))
                        mm.append((psS, onescol[:], pb[:, sl], sl, True))
                    if t == nj - 1:
                        mm.sort(key=lambda x: not x[4])
                    for dst, st, mv, sl, is_s in mm:
                        if is_s:
                            nc.tensor.matmul(psS[0:1, sl], st, mv,
                                             start=(t == 0),
                                             stop=(t == nj - 1))
                        else:
                            nc.tensor.matmul(psAT[:, sl], st, mv,
                                             start=(t == 0),
                                             stop=(t == nj - 1))

                for t in range(njl):
                    att_tile(t)
                if l > 0:
                    # import the partner's h on DVE (idle here; all local
                    # tile work is already ahead of it in the queue):
                    # remote = (hg0 + hg1) - local, exact in f32
                    hsum = hgp.tile([P, NS], F32, name=f"hsum{l}",
                                    tag="hsum")
                    nc.vector.tensor_tensor(hsum[:], hg0[:], hg1[:], OP.add)
                    nc.vector.tensor_tensor(hT[:, NS:N], hsum[:],
                                            hT[:, 0:NS], OP.subtract)
                    prep_group(2)
                    prep_group(3)
                for t in range(njl, nj):
                    att_tile(t)

                # ---- normalize + residual (transposed layout) ----
                s_row = smallp.tile([1, NS], F32, name=f"srow{l}",
                                    tag="srow")
                nc.vector.tensor_copy(s_row[:], psS[:])
                scol = pe_pool.tile([P, nit], F32, name=f"scol{l}",
                                    tag="pe")
                for k in range(nit):
                    nc.tensor.transpose(scol[:, k:k + 1],
                                        s_row[0:1, k * P:(k + 1) * P],
                                        identf[0:1, 0:1])
                rs = smallp.tile([P, nit], F32, name=f"rs{l}", tag="rs")
                nc.vector.reciprocal(rs[:], scol[:])
                rsb = smallp.tile([P, nit], BF16, name=f"rsb{l}", tag="rsb")
                nc.vector.tensor_copy(rsb[:], rs[:])
                r_row = smallp.tile([1, NS], BF16, name=f"rrow{l}",
                                    tag="rrow")
                for ch in range(nch):
                    rp = pe_pool.tile([1, 512], BF16, name=f"rp{l}_{ch}",
                                      tag="pe")
                    for k in range(4):
                        it = ch * 4 + k
                        nc.tensor.transpose(rp[0:1, k * P:(k + 1) * P],
                                            rsb[:, it:it + 1], identb[:])
                    nc.vector.tensor_copy(r_row[0:1,
                                                ch * 512:(ch + 1) * 512],
                                          rp[:])
                xTsb_new = xtp.tile([P, NS], BF16, name=f"xTsb{l + 1}",
                                    tag="xTsb")
                tmps = []
                for ch in range(nch):
                    sl = slice(ch * 512, (ch + 1) * 512)
                    bps = pe_pool.tile([P, 512], F32, name=f"bcr{l}_{ch}",
                                       tag="pe")
                    nc.tensor.matmul(bps[:], onesrowb[:], r_row[0:1, sl],
                                     start=True, stop=True)
                    rbc = smallp.tile([P, 512], BF16, name=f"rbc{l}_{ch}",
                                      tag=f"rbc{ch}")
                    nc.vector.tensor_copy(rbc[:], bps[:])
                    tmp = qp.tile([P, 512], F32, name=f"tmp{l}_{ch}",
                                  tag=f"tmp{ch}")
                    nc.vector.tensor_tensor(tmp[:], psAT[:, sl], rbc[:],
                                            OP.mult)
                    nc.vector.tensor_tensor(xTsb_new[:, sl], tmp[:],
                                            xTs[:, sl], OP.add)
                    tmps.append(tmp)
                if not last:
                    xTs_new = xtp.tile([P, NS], F32, name=f"xTs{l + 1}",
                                       tag="xTs")

                    def resid(xTs_new=xTs_new, tmps=tmps, xTs=xTs):
                        for ch in range(nch):
                            sl = slice(ch * 512, (ch + 1) * 512)
                            nc.gpsimd.tensor_tensor(xTs_new[:, sl],
                                                    tmps[ch][:],
                                                    xTs[:, sl], OP.add)
                    pending.append(resid)
                    xTs = xTs_new
                xTsb = xTsb_new

            # ---- final linear: outT = relu(WtT^T @ xTsb + bt) ----
            for g in range(nH):
                for ch in range(nch):
                    sl = slice(ch * 512, (ch + 1) * 512)
                    ps = pe_pool.tile([P, 512], F32, name=f"ops{g}_{ch}",
                                      tag="pe")
                    nc.tensor.matmul(ps[:], WtTt[:, g * P:(g + 1) * P],
                                     xTsb[:, sl], start=True, stop=True)
                    oc = ocp.tile([P, 512], F32, name=f"oc{g}_{ch}",
                                  tag="oc")
                    nc.vector.tensor_scalar(oc[:], ps[:], btpt[:, g:g + 1],
                                            0.0, OP.add, OP.max)
                    nc.sync.dma_start(
                        out_ext.ap()[g * P:(g + 1) * P, sl], oc[:])

    if legalize:
        _legalize_waits(nc)
    return nc


def make_in_maps(x, adj, Ws, bs, avs, Wt, bt, num_cores, NS):
    """Per-core input dicts. Core c -> (graph c//2, row-half c%2).
    Column (j) layout is [local | remote] per core."""
    B, N, D = x.shape
    H = Wt.shape[0]
    nH = H // P
    x = np.ascontiguousarray(x, np.float32)
    adj = np.asarray(adj)
    shared = {"WtT": np.ascontiguousarray(
                  np.asarray(Wt, np.float32).T).astype(ml_dtypes.bfloat16),
              "btp": np.ascontiguousarray(
                  np.asarray(bt, np.float32).reshape(nH, P).T)}
    for l, (W, b, a) in enumerate(zip(Ws, bs, avs)):
        shared[f"WT{l}"] = np.ascontiguousarray(
            np.asarray(W, np.float32).T).astype(ml_dtypes.bfloat16)
        shared[f"bv{l}"] = np.ascontiguousarray(
            np.asarray(b, np.float32).reshape(D, 1))
        shared[f"av{l}"] = np.ascontiguousarray(
            np.stack([np.asarray(a, np.float32)[:D, 0],
                      np.asarray(a, np.float32)[D:, 0]],
                     axis=1)).astype(ml_dtypes.bfloat16)
    in_maps = []
    for c in range(num_cores):
        b, s = c // 2, c % 2
        m = dict(shared)
        xT = np.ascontiguousarray(x[b].T)
        loc = slice(s * NS, (s + 1) * NS)
        rem = slice((1 - s) * NS, (2 - s) * NS)
        m["xTs"] = np.ascontiguousarray(xT[:, loc])
        m["xTsb"] = m["xTs"].astype(ml_dtypes.bfloat16)
        m["xTb"] = np.concatenate([xT[:, loc], xT[:, rem]],
                                  axis=1).astype(ml_dtypes.bfloat16)
        adjT = adj[b, loc, :].T.astype(ml_dtypes.bfloat16)  # [N j, NS i]
        m["maskTb"] = np.ascontiguousarray(
            np.concatenate([adjT[loc, :], adjT[rem, :]], axis=0))
        in_maps.append(m)
    return in_maps


_NC_CACHE = {}


def kernel(x, adj, W0, b0, W1, b1, W2, b2, a0, a1, a2, Wt, bt):
    B, N, D = 4, 2048, 128
    H = 256
    NUM_CORES = 8
    NS = N // 2
    pair_groups = [[2 * i, 2 * i + 1] for i in range(NUM_CORES // 2)]

    key = (N, NS, D, H, NUM_CORES)
    if key not in _NC_CACHE:
        _NC_CACHE[key] = build_gat_nc(N, NS, D, H, NUM_CORES, pair_groups)
    nc = _NC_CACHE[key]

    in_maps = make_in_maps(np.asarray(x), np.asarray(adj),
                           [W0, W1, W2], [b0, b1, b2], [a0, a1, a2],
                           np.asarray(Wt), np.asarray(bt), NUM_CORES, NS)
    res = run_bass_kernel_spmd(nc, in_maps, list(range(NUM_CORES))).results
    out = np.empty((B, N, H), np.float32)
    for c in range(NUM_CORES):
        b, s = c // 2, c % 2
        out[b, s * NS:(s + 1) * NS, :] = res[c]["outT_s"].T
    return out
